# revision 9
# baseline (speedup 1.0000x reference)
"""AffinityLoss (torchdistill) Trainium2 kernel.

loss = mean_b [ sum_c sqrt(2 - 2*cos^2(s_bc, t_bc)) / HW ]

with s_bc, t_bc the HW-dim spatial vectors of channel c of sample b.
cos(s, t) = <s,t> / (||s|| ||t||), so per (b, c) we only need the three
dot products ss, tt, st over the 16384-element spatial dim.

Sharding: data-parallel over the batch dim B=8 -> one sample per
NeuronCore (8 cores). Per core, channels live on SBUF partitions
(2 chunks of 128) and the spatial dim is tiled along the free dim.

The kernel is a pure streaming 3-accumulator reduction, engine-split so
every engine stays well under the 433 GB/s DMA feed rate (GpSimd cannot
do a fused square+accumulate on this compiler build, so the square work
is load-balanced across ACT and DVE instead):
  - ScalarE (ACT):  Square(s) accum -> ss; t^2 for even cols  (~66%)
  - VectorE (DVE):  stt(s*t) accum -> st; t^2 for odd cols    (~67%)
Both write their full-size `out` through a stride-0 broadcast dummy
(no scratch SBUF, no write bandwidth). Per spatial tile each engine
drops one fp32 partial per channel into its column of a [128, 3*NCOL]
accumulator tile; a single tiny DMA ships it to HBM at the end and the
host finishes the per-channel closed form sqrt(2-2*st^2/(ss*tt)) in
float64 (2048 channels/core -- negligible).

The spatial schedule tapers (8192 ... 1024) so the post-last-DMA tail
is short (below 1024 the fixed ~0.3us accumulator-read per op makes the
engines oversubscribed, so the taper stops there). The last column's
t^2 runs on ACT, which squared s while t was still streaming.
"""

import numpy as np

import concourse.bacc as bacc
import concourse.tile as tile
from concourse import mybir
from concourse.bass_utils import run_bass_kernel_spmd

B, C, H, W = 8, 256, 128, 128
HW = H * W           # 16384 spatial elements per channel
P = 128              # SBUF partitions
NCORES = 8

F = 8192             # max spatial tile width (per-DMA: 128 x 8192 x 4B = 4 MiB)

# Per-channel-chunk spatial tile widths. cc0 streams wide; the last cc
# tapers so the compute tail after the final DMA is short.
WIDTHS_CC0 = [8192, 8192]
WIDTHS_CC1 = [8192, 4096, 2048, 1536, 512]


def _tile_schedule():
    """(cc, offset, width, col) list + per-cc col ranges."""
    sched = []
    col = 0
    col_ranges = []
    for cc, widths in enumerate((WIDTHS_CC0, WIDTHS_CC1)):
        assert sum(widths) == HW
        c0 = col
        off = 0
        for w in widths:
            sched.append((cc, off, w, col))
            off += w
            col += 1
        col_ranges.append((c0, col))
    return sched, col_ranges


_SCHED, _COL_RANGES = _tile_schedule()
NCOL = len(_SCHED)


def build_program(ncores=NCORES):
    f32 = mybir.dt.float32
    CC = C // P          # channel chunks (channels on partitions)

    nc = bacc.Bacc("TRN2", target_bir_lowering=False, debug=False,
                   num_devices=ncores)
    s_d = nc.dram_tensor("student", [C, HW], f32, kind="ExternalInput").ap()
    t_d = nc.dram_tensor("teacher", [C, HW], f32, kind="ExternalInput").ap()
    out_d = nc.dram_tensor("out", [P, 3 * NCOL], f32, kind="ExternalOutput").ap()

    last_col = NCOL - 1

    with tile.TileContext(nc) as tc:
        with (
            tc.tile_pool(name="io", bufs=3) as io,
            tc.tile_pool(name="small", bufs=1) as small,
        ):
            acc = small.tile([P, 3 * NCOL], f32)   # [ss | tt | st] columns
            dummy_act = small.tile([P, 1], f32)
            dummy_dve = small.tile([P, 1], f32)

            # Early touch: ACT Square rides in an activation table set --
            # touching it first makes the ~1.3us table load overlap the DMA
            # ramp instead of serializing after the first tile lands.
            nc.vector.memset(dummy_act, 1.0)
            nc.vector.memset(dummy_dve, 1.0)
            nc.scalar.activation(
                out=dummy_act, in_=dummy_act,
                func=mybir.ActivationFunctionType.Square,
            )

            def square(engine, dummy, src, accum):
                if engine == "scalar":
                    nc.scalar.activation(
                        out=dummy.broadcast_to(src.shape), in_=src,
                        func=mybir.ActivationFunctionType.Square,
                        accum_out=accum,
                    )
                else:
                    nc.vector.scalar_tensor_tensor(
                        out=dummy.broadcast_to(src.shape),
                        in0=src, scalar=1.0, in1=src,
                        op0=mybir.AluOpType.mult, op1=mybir.AluOpType.mult,
                        accum_out=accum,
                    )

            for cc, off, w, col in _SCHED:
                s_tile = io.tile([P, F], f32, tag="s")
                nc.sync.dma_start(
                    out=s_tile[:, :w],
                    in_=s_d[cc * P:(cc + 1) * P, off:off + w],
                )
                t_tile = io.tile([P, F], f32, tag="t")
                nc.sync.dma_start(
                    out=t_tile[:, :w],
                    in_=t_d[cc * P:(cc + 1) * P, off:off + w],
                )

                ss_col = acc[:, col:col + 1]
                tt_col = acc[:, NCOL + col:NCOL + col + 1]
                st_col = acc[:, 2 * NCOL + col:2 * NCOL + col + 1]

                square("scalar", dummy_act, s_tile[:, :w], ss_col)
                # Alternate t^2 between ACT and DVE so both sit at ~66%
                # of capacity; the last col goes to ACT, which finished
                # its s^2 while t was still streaming in.
                if col % 2 == 0 or col == last_col:
                    square("scalar", dummy_act, t_tile[:, :w], tt_col)
                else:
                    square("vector", dummy_dve, t_tile[:, :w], tt_col)

                # NOTE: tensor_tensor_reduce wedges the exec unit on this
                # runtime build; scalar_tensor_tensor + accum_out is the
                # same single-pass fused multiply-reduce on the DVE.
                nc.vector.scalar_tensor_tensor(
                    out=dummy_dve.broadcast_to(s_tile[:, :w].shape),
                    in0=s_tile[:, :w],
                    scalar=1.0,
                    in1=t_tile[:, :w],
                    op0=mybir.AluOpType.mult,
                    op1=mybir.AluOpType.mult,
                    accum_out=st_col,
                )

            nc.sync.dma_start(out=out_d, in_=acc)

    nc.finalize()
    return nc


_PROGRAM = None


def _get_program():
    global _PROGRAM
    if _PROGRAM is None:
        _PROGRAM = build_program()
    return _PROGRAM


def _host_epilogue(acc_list) -> float:
    """acc_list: per-core [128, 3*NCOL] fp32. Finish the per-channel
    closed form in float64 and return the scalar loss."""
    total = 0.0
    for a in acc_list:
        a = np.asarray(a, dtype=np.float64)
        ss_cols = a[:, 0:NCOL]
        tt_cols = a[:, NCOL:2 * NCOL]
        st_cols = a[:, 2 * NCOL:3 * NCOL]
        for c0, c1 in _COL_RANGES:
            ss = ss_cols[:, c0:c1].sum(axis=1)
            tt = tt_cols[:, c0:c1].sum(axis=1)
            st = st_cols[:, c0:c1].sum(axis=1)
            cos2 = (st * st) / (ss * tt)
            w = np.sqrt(np.clip(2.0 - 2.0 * cos2, 0.0, None))
            total += float(w.sum())
    return total / (HW * B)


def kernel(student: np.ndarray, teacher: np.ndarray) -> np.ndarray:
    s = np.ascontiguousarray(np.asarray(student, dtype=np.float32)).reshape(B, C, HW)
    t = np.ascontiguousarray(np.asarray(teacher, dtype=np.float32)).reshape(B, C, HW)

    nc = _get_program()
    in_maps = [{"student": s[i], "teacher": t[i]} for i in range(NCORES)]
    results = run_bass_kernel_spmd(nc, in_maps, list(range(NCORES))).results

    total = _host_epilogue([results[i]["out"] for i in range(NCORES)])
    return np.asarray(total, dtype=np.float32)
